# revision 26
# baseline (speedup 1.0000x reference)
"""Causal self-attention (B=8, T=2048, C=256, H=8, D=32) on 8 trn2 NeuronCores.

Sharding: pure data-parallel over batch - core b computes batch element b
end-to-end (no collectives).

Host-side prep (free): x is transposed to xT [C, T] and cast to bf16, weights
cast to bf16, so the kernel needs no on-chip transposes or weight casts.

Per-core kernel (all matmul inputs bf16, fp32 PSUM, softmax math in fp32):
  1. qT,kT [256,T] bf16: PE matmuls lhsT=w_attn slice, rhs=xT.
     v_aug [128, 8*33] per key-tile: per-head 32 v-cols + ones column (the
     ones column makes the PV matmul accumulate the softmax denominator).
  2. Attention in S^T layout, head-PAIRS (hg in 0..3), m-chunks of W=512
     queries, n-tiles of 128 keys. Per (mc,hg,nn):
       scores: 2 row-tiled K=32 matmuls -> s_ps [128, 2, W] (2 PSUM banks,
         double buffered); diagonal tiles slice columns >= off.
       exp: ONE ACT instr (scale=1/sqrt(D) fused) PSUM fp32 -> pt bf16.
       mask: DVE triangular multiply on diagonal blocks.
       PV: 2 col-tiled M=33 matmuls accumulating y_aug^T [33, W] per head
         into y_ps (1 bank, double buffered across hg).
     EMISSION IS SOFTWARE-PIPELINED: scores(nn+1) is emitted BEFORE
     exp/PV(nn) so the FIFO Tensor queue never head-blocks on the ACT
     dependency (this was the baseline's serialization bug).
  3. Drain: y rows -> yaug bf16 tiles; denominator rows gathered into one
     [32, W] tile per m-chunk column range; ONE reciprocal_approx_fast for
     all 32 (mc,hg,i) rows; gpsimd partition_broadcast + DVE multiply
     -> normalized yT [256, T] bf16.
  4. proj: out = y @ w_proj -> PSUM -> SBUF -> DMA to DRAM.
"""

import numpy as np
from contextlib import ExitStack

import concourse.bass as bass
import concourse.bacc as bacc
import concourse.mybir as mybir
import concourse.tile as tile
from concourse.bass import ds
from concourse.bass_utils import run_bass_kernel_spmd
from concourse.masks import make_upper_triangular

FP32 = mybir.dt.float32
BF16 = mybir.dt.bfloat16

C = 256
H = 8
D = 32
DA = D + 1  # 33: v columns + denominator ones column
N_CORES = 8
SCALE = 1.0 / float(np.sqrt(np.float32(D)))


def build_body(ctx: ExitStack, tc: tile.TileContext, xT, wa, wp, out, T: int,
               taps=None):
    nc = tc.nc
    TT = T // 128              # number of 128-key tiles
    W = min(512, T)            # m-chunk width (queries)
    MCN = T // W               # number of m-chunks
    WT = W // 128              # n-tiles per m-chunk width

    const = ctx.enter_context(tc.tile_pool(name="const", bufs=1))
    wpool = ctx.enter_context(tc.tile_pool(name="wpool", bufs=1))
    xTp = ctx.enter_context(tc.tile_pool(name="xTp", bufs=1))
    qkTp = ctx.enter_context(tc.tile_pool(name="qkTp", bufs=1))
    vaugp = ctx.enter_context(tc.tile_pool(name="vaugp", bufs=TT))
    ptp = ctx.enter_context(tc.tile_pool(name="ptp", bufs=3))
    nrmp = ctx.enter_context(tc.tile_pool(name="nrmp", bufs=4))
    ytp = ctx.enter_context(tc.tile_pool(name="ytp", bufs=1))
    ostp = ctx.enter_context(tc.tile_pool(name="ostp", bufs=3))

    # PSUM budget (8 banks of [128, 2KB]):
    #   ps_s:  [128, 2, W] fp32 = 2 banks x 2 bufs = 4 banks
    #   ps_y:  [128, W] fp32    = 1 bank  x 3 bufs = 3 banks
    #     (3-deep so the normalize chain - reciprocal included - of head-pair
    #      hg never stalls hg+2's PV accumulation via y_ps reuse)
    #   ps_sm: [128, <=512] fp32 <= 1 bank x 1 buf = 1 bank
    ps_s = ctx.enter_context(tc.tile_pool(name="ps_s", bufs=2, space="PSUM"))
    ps_y = ctx.enter_context(tc.tile_pool(name="ps_y", bufs=3, space="PSUM"))
    ps_sm = ctx.enter_context(tc.tile_pool(name="ps_sm", bufs=1, space="PSUM"))

    # --- constants: triangular keep-mask (key <= query), replicated per slot
    tri = const.tile([128, 128], BF16)
    make_upper_triangular(nc, tri[:], val=1.0, diag=True)
    tri2 = const.tile([128, 2, 128], BF16)
    for i in range(2):
        nc.vector.tensor_copy(tri2[:, i, :], tri[:])
    # ones row for the K=1 broadcast matmul in the normalize step
    ones1 = const.tile([1, D], BF16)
    nc.gpsimd.memset(ones1[:], 1.0)

    # --- weights: already bf16 from host ---
    wa_bf = []
    wp_bf = []
    for k in range(2):
        wab = wpool.tile([128, 3 * C], BF16, name=f"wa_bf{k}")
        nc.sync.dma_start(wab[:], wa[128 * k:128 * (k + 1), :])
        wa_bf.append(wab)
        wpb = wpool.tile([128, C], BF16, name=f"wp_bf{k}")
        nc.sync.dma_start(wpb[:], wp[128 * k:128 * (k + 1), :])
        wp_bf.append(wpb)

    # --- xT: already transposed + bf16 from host; chunked DMA loads ---
    xTt = [xTp.tile([128, T], BF16, name=f"xT{k}") for k in range(2)]
    for k in range(2):
        for tck in range(MCN):
            nc.sync.dma_start(xTt[k][:, W * tck:W * (tck + 1)],
                              xT[128 * k:128 * (k + 1), W * tck:W * (tck + 1)])

    # --- qT, kT [256, T] (2 f-tiles each) + v_aug per key tile ---
    # tck-major order: everything attention chunk 0 needs is produced
    # first, so the attention loop starts ~10us earlier and the PE never
    # sees a long idle window at the pre->attention transition.
    qkT = [qkTp.tile([128, T], BF16, name=f"qkT{f}") for f in range(4)]
    vaug = [None] * TT
    for tck in range(MCN):
        for f in range(4):
            ps = ps_sm.tile([128, W], FP32, name="qk_ps", tag="sm")
            for k in range(2):
                nc.tensor.matmul(
                    ps[:],
                    wa_bf[k][:, 128 * f:128 * (f + 1)],
                    xTt[k][:, W * tck:W * (tck + 1)],
                    start=(k == 0),
                    stop=(k == 1),
                )
            nc.vector.tensor_copy(qkT[f][:, W * tck:W * (tck + 1)], ps[:])
        for tt in range(WT * tck, WT * (tck + 1)):
            ps = ps_sm.tile([128, C], FP32, name="v_ps", tag="sm")
            for k in range(2):
                nc.tensor.matmul(
                    ps[:],
                    xTt[k][:, 128 * tt:128 * (tt + 1)],
                    wa_bf[k][:, 2 * C:3 * C],
                    start=(k == 0),
                    stop=(k == 1),
                )
            va = vaugp.tile([128, H * DA], BF16, name="va")
            nc.gpsimd.memset(va[:], 1.0)
            nc.vector.tensor_copy(
                va[:].rearrange("p (h d) -> p h d", h=H)[:, :, 0:D],
                ps[:].rearrange("p (h d) -> p h d", h=H),
            )
            vaug[tt] = va

    if taps is not None:
        for f in range(4):
            nc.gpsimd.dma_start(taps[f"qkT{f}"], qkT[f][:])
        nc.gpsimd.dma_start(taps["vaug0"], vaug[0][:])

    # --- attention: head-pairs hg (heads 2hg, 2hg+1), software-pipelined ---
    yT = [ytp.tile([128, T], BF16, name=f"yT{g}") for g in range(2)]
    for mc in range(MCN):
        for hg in range(4):
            nn_count = WT * (mc + 1)
            nn_last = nn_count - 1
            y_ps = ps_y.tile([128, W], FP32, name="y_ps", tag="y")
            s_list = [None] * nn_count
            pt_list = [None] * nn_count

            def emit_scores(nn):
                j = nn - WT * mc
                off = max(0, 128 * j)
                s_ps = ps_s.tile([128, 2, W], FP32, name="s_ps", tag="s")
                s_list[nn] = (s_ps, off)
                for i in range(2):
                    h = 2 * hg + i
                    fq, r = h // 4, 32 * (h % 4)
                    nc.tensor.matmul(
                        s_ps[:, i, ds(off, W - off)],
                        qkT[2 + fq][ds(r, 32), 128 * nn:128 * (nn + 1)],
                        qkT[fq][ds(r, 32), W * mc + off:W * (mc + 1)],
                        start=True,
                        stop=True,
                        tile_position=(r, 0),
                        skip_group_check=True,
                    )

            def emit_softmax_pv(nn):
                s_ps, off = s_list[nn]
                pt = ptp.tile([128, 2, W], BF16, name="pt")
                pt_list[nn] = pt
                nc.scalar.activation(
                    pt[:, :, ds(off, W - off)],
                    s_ps[:, :, ds(off, W - off)],
                    mybir.ActivationFunctionType.Exp,
                    scale=SCALE,
                )
                if nn - WT * mc >= 0:
                    # triangular mask on the idle GpSimd engine - keeps the
                    # DVE out of the exp->PV critical hop
                    blk = pt[:, :, ds(off, 128)]
                    nc.gpsimd.tensor_mul(blk, blk, tri2[:, :, 0:128])
                for i in range(2):
                    h = 2 * hg + i
                    nc.tensor.matmul(
                        y_ps[ds(64 * i, DA), ds(off, W - off)],
                        vaug[nn][:, DA * h:DA * (h + 1)],
                        pt[:, i, ds(off, W - off)],
                        start=(nn == 0),
                        stop=(nn == nn_last),
                        tile_position=(0, 64 * i),
                        skip_group_check=True,
                    )

            # software pipeline: scores(nn+1) ahead of exp/PV(nn) so the
            # Tensor FIFO never blocks behind the ACT dependency
            emit_scores(0)
            for nn in range(nn_count):
                if nn + 1 < nn_count:
                    emit_scores(nn + 1)
                emit_softmax_pv(nn)
                if taps is not None and mc == 0 and hg == 0 and nn == 0:
                    nc.gpsimd.dma_start(taps["pt00"], pt_list[0][:])

            # normalize straight out of PSUM: reciprocal of the denominator
            # row, K=1 ones-matmul partition-broadcast into PSUM, one
            # multiply into yT.  (walrus requires SBUF operands of a
            # tensor_tensor to share the start partition; both value inputs
            # here are PSUM, so only yT counts.)
            # (custom-DVE reciprocal must read SBUF, not PSUM - copy first)
            # denominator rows parked at partitions 0 and 32 (engine APs must
            # start at a multiple of 32); one approx reciprocal covers both
            g = hg // 2
            den_sb = nrmp.tile([64, W], FP32, name="den_sb", tag="den")
            for i in range(2):
                nc.vector.tensor_copy(den_sb[ds(32 * i, 1), :],
                                      y_ps[ds(64 * i + D, 1), :])
            # one batched reciprocal for both heads (rows 1-31/33-63 unused;
            # custom-DVE approx ops race under Tile, plain InstReciprocal
            # is dependable)
            rcp = nrmp.tile([64, W], FP32, name="rcp", tag="rcp")
            nc.vector.reciprocal(rcp[:], den_sb[:])
            rcp_bf = [nrmp.tile([1, W], BF16, name=f"rcp_bf{i}",
                                tag=f"rcpbf{i}") for i in range(2)]
            for i in range(2):
                nc.vector.tensor_copy(rcp_bf[i][:], rcp[ds(32 * i, 1), :])
            for i in range(2):
                row = 64 * (hg % 2) + 32 * i
                bc_ps = ps_sm.tile([D, W], FP32, name="bc_ps", tag="sm")
                nc.tensor.matmul(bc_ps[:], ones1[:], rcp_bf[i][:],
                                 start=True, stop=True)
                bcast = nrmp.tile([D, W], BF16, name="bcast", tag="bc")
                nc.vector.tensor_copy(bcast[:], bc_ps[:])
                nc.vector.tensor_mul(
                    yT[g][ds(row, D), W * mc:W * (mc + 1)],
                    y_ps[ds(64 * i, D), :],
                    bcast[:],
                )

        # projection for this m-chunk's t-tiles: its yT columns are final
        # once all four head-pairs are normalized; interleaving it here
        # shrinks the serial tail and feeds the PE at chunk boundaries
        for tt in range(WT * mc, WT * (mc + 1)):
            ps = ps_sm.tile([128, C], FP32, name="pj_ps", tag="sm")
            for g in range(2):
                nc.tensor.matmul(
                    ps[:],
                    yT[g][:, 128 * tt:128 * (tt + 1)],
                    wp_bf[g][:],
                    start=(g == 0),
                    stop=(g == 1),
                )
            ost = ostp.tile([128, C], FP32, name="ost")
            nc.vector.tensor_copy(ost[:], ps[:])
            # SWDGE: out-store waits are executed by Q7 ucode (no 1-wait cap)
            nc.gpsimd.dma_start(out[128 * tt:128 * (tt + 1), :], ost[:])

    if taps is not None:
        for g in range(2):
            nc.gpsimd.dma_start(taps[f"yT{g}"], yT[g][:])


def build_nc(T: int = 2048, debug: bool = False) -> bass.Bass:
    # Bacc (not raw Bass): its compile() pass legalizes multi-sem waits via
    # event semaphores - walrus only accepts one sem wait per instruction.
    nc = bacc.Bacc("TRN2", target_bir_lowering=False, debug=False,
                   num_devices=N_CORES)
    xT_d = nc.dram_tensor("xT", [C, T], BF16, kind="ExternalInput")
    wa_d = nc.dram_tensor("w_attn", [C, 3 * C], BF16, kind="ExternalInput")
    wp_d = nc.dram_tensor("w_proj", [C, C], BF16, kind="ExternalInput")
    out_d = nc.dram_tensor("out", [T, C], FP32, kind="ExternalOutput")
    taps = None
    if debug:
        taps = {}
        for f in range(4):
            taps[f"qkT{f}"] = nc.dram_tensor(
                f"tap_qkT{f}", [128, T], BF16, kind="ExternalOutput").ap()
        taps["vaug0"] = nc.dram_tensor(
            "tap_vaug0", [128, H * DA], BF16, kind="ExternalOutput").ap()
        taps["pt00"] = nc.dram_tensor(
            "tap_pt00", [128, 2 * min(512, T)], BF16,
            kind="ExternalOutput").ap()
        for g in range(2):
            taps[f"yT{g}"] = nc.dram_tensor(
                f"tap_yT{g}", [128, T], BF16, kind="ExternalOutput").ap()
    with tile.TileContext(nc) as tc:
        with ExitStack() as ctx:
            build_body(ctx, tc, xT_d.ap(), wa_d.ap(), wp_d.ap(), out_d.ap(), T,
                       taps=taps)
    nc.compile()
    return nc


_NC_CACHE: dict[int, bass.Bass] = {}


def _get_nc(T: int) -> bass.Bass:
    if T not in _NC_CACHE:
        _NC_CACHE[T] = build_nc(T)
    return _NC_CACHE[T]


def kernel(x: np.ndarray, w_attn: np.ndarray, w_proj: np.ndarray,
           _return_raw: bool = False, **run_kwargs) -> np.ndarray:
    import ml_dtypes
    B, T, C_ = x.shape
    assert B == N_CORES and C_ == C
    nc = _get_nc(T)
    wa = np.ascontiguousarray(np.asarray(w_attn, np.float32).astype(
        ml_dtypes.bfloat16))
    wp = np.ascontiguousarray(np.asarray(w_proj, np.float32).astype(
        ml_dtypes.bfloat16))
    xb = np.asarray(x, np.float32).astype(ml_dtypes.bfloat16)
    in_maps = [
        {"xT": np.ascontiguousarray(xb[b].T), "w_attn": wa, "w_proj": wp}
        for b in range(B)
    ]
    res = run_bass_kernel_spmd(nc, in_maps, list(range(N_CORES)), **run_kwargs)
    if _return_raw:
        return res
    out = np.stack([res.results[b]["out"] for b in range(B)])
    return out.astype(np.float32)


# revision 29
# speedup vs baseline: 1.0399x; 1.0399x over previous
"""Causal self-attention (B=8, T=2048, C=256, H=8, D=32) on 8 trn2 NeuronCores.

Sharding: pure data-parallel over batch - core b computes batch element b
end-to-end (no collectives).

Host-side prep (free): x is transposed to xT [C, T] and cast to bf16, weights
cast to bf16, so the kernel needs no on-chip transposes or weight casts.

Per-core kernel (all matmul inputs bf16, fp32 PSUM, softmax math in fp32):
  1. qT,kT [256,T] bf16: PE matmuls lhsT=w_attn slice, rhs=xT.
     v_aug [128, 8*33] per key-tile: per-head 32 v-cols + ones column (the
     ones column makes the PV matmul accumulate the softmax denominator).
  2. Attention in S^T layout, head-PAIRS (hg in 0..3), m-chunks of W=512
     queries, n-tiles of 128 keys. Per (mc,hg,nn):
       scores: 2 row-tiled K=32 matmuls -> s_ps [128, 2, W] (2 PSUM banks,
         double buffered); diagonal tiles slice columns >= off.
       exp: ONE ACT instr (scale=1/sqrt(D) fused) PSUM fp32 -> pt bf16.
       mask: DVE triangular multiply on diagonal blocks.
       PV: 2 col-tiled M=33 matmuls accumulating y_aug^T [33, W] per head
         into y_ps (1 bank, double buffered across hg).
     EMISSION IS SOFTWARE-PIPELINED: scores(nn+1) is emitted BEFORE
     exp/PV(nn) so the FIFO Tensor queue never head-blocks on the ACT
     dependency (this was the baseline's serialization bug).
  3. Normalize per (mc,hg): denominator rows copied to partitions 0/32 of
     an SBUF tile, one batched DVE reciprocal, K=1 ones-matmul broadcast
     into PSUM, DVE multiply -> normalized yT [256, T] bf16.  (Masks run
     on gpsimd so the DVE FIFO never head-blocks the exp->PV hop.)
  4. proj per m-chunk (interleaved): out = y @ w_proj -> PSUM -> SBUF ->
     DMA to DRAM.
"""

import numpy as np
from contextlib import ExitStack

import concourse.bass as bass
import concourse.bacc as bacc
import concourse.mybir as mybir
import concourse.tile as tile
from concourse.bass import ds
from concourse.bass_utils import run_bass_kernel_spmd
from concourse.masks import make_upper_triangular

FP32 = mybir.dt.float32
BF16 = mybir.dt.bfloat16

C = 256
H = 8
D = 32
DA = D + 1  # 33: v columns + denominator ones column
N_CORES = 8
SCALE = 1.0 / float(np.sqrt(np.float32(D)))


def build_body(ctx: ExitStack, tc: tile.TileContext, xT, wa, wp, out, T: int,
               taps=None):
    nc = tc.nc
    TT = T // 128              # number of 128-key tiles
    W = min(512, T)            # m-chunk width (queries)
    MCN = T // W               # number of m-chunks
    WT = W // 128              # n-tiles per m-chunk width

    const = ctx.enter_context(tc.tile_pool(name="const", bufs=1))
    wpool = ctx.enter_context(tc.tile_pool(name="wpool", bufs=1))
    xTp = ctx.enter_context(tc.tile_pool(name="xTp", bufs=1))
    qkTp = ctx.enter_context(tc.tile_pool(name="qkTp", bufs=1))
    vaugp = ctx.enter_context(tc.tile_pool(name="vaugp", bufs=TT))
    ptp = ctx.enter_context(tc.tile_pool(name="ptp", bufs=3))
    nrmp = ctx.enter_context(tc.tile_pool(name="nrmp", bufs=4))
    ytp = ctx.enter_context(tc.tile_pool(name="ytp", bufs=1))
    ostp = ctx.enter_context(tc.tile_pool(name="ostp", bufs=3))

    # PSUM budget (8 banks of [128, 2KB]):
    #   ps_s:  [128, 2, W] fp32 = 2 banks x 2 bufs = 4 banks
    #   ps_y:  [128, W] fp32    = 1 bank  x 2 bufs = 2 banks
    #   ps_sm: [128, <=512] fp32 <= 1 bank x 2 bufs = 2 banks
    ps_s = ctx.enter_context(tc.tile_pool(name="ps_s", bufs=2, space="PSUM"))
    ps_y = ctx.enter_context(tc.tile_pool(name="ps_y", bufs=2, space="PSUM"))
    ps_sm = ctx.enter_context(tc.tile_pool(name="ps_sm", bufs=2, space="PSUM"))

    # --- constants: triangular keep-mask (key <= query), replicated per slot
    tri = const.tile([128, 128], BF16)
    make_upper_triangular(nc, tri[:], val=1.0, diag=True)
    tri2 = const.tile([128, 2, 128], BF16)
    for i in range(2):
        nc.vector.tensor_copy(tri2[:, i, :], tri[:])
    # ones row for the K=1 broadcast matmul in the normalize step
    ones1 = const.tile([1, D], BF16)
    nc.gpsimd.memset(ones1[:], 1.0)

    # --- weights: already bf16 from host ---
    wa_bf = []
    wp_bf = []
    for k in range(2):
        wab = wpool.tile([128, 3 * C], BF16, name=f"wa_bf{k}")
        nc.sync.dma_start(wab[:], wa[128 * k:128 * (k + 1), :])
        wa_bf.append(wab)
        wpb = wpool.tile([128, C], BF16, name=f"wp_bf{k}")
        nc.sync.dma_start(wpb[:], wp[128 * k:128 * (k + 1), :])
        wp_bf.append(wpb)

    # --- xT: already transposed + bf16 from host; chunked DMA loads ---
    xTt = [xTp.tile([128, T], BF16, name=f"xT{k}") for k in range(2)]
    for k in range(2):
        for tck in range(MCN):
            nc.sync.dma_start(xTt[k][:, W * tck:W * (tck + 1)],
                              xT[128 * k:128 * (k + 1), W * tck:W * (tck + 1)])

    # --- qT, kT [256, T] (2 f-tiles each) + v_aug per key tile ---
    # tck-major order: everything attention chunk 0 needs is produced
    # first, so the attention loop starts ~10us earlier and the PE never
    # sees a long idle window at the pre->attention transition.
    qkT = [qkTp.tile([128, T], BF16, name=f"qkT{f}") for f in range(4)]
    vaug = [None] * TT
    for tck in range(MCN):
        for f in range(4):
            ps = ps_sm.tile([128, W], FP32, name="qk_ps", tag="sm")
            for k in range(2):
                nc.tensor.matmul(
                    ps[:],
                    wa_bf[k][:, 128 * f:128 * (f + 1)],
                    xTt[k][:, W * tck:W * (tck + 1)],
                    start=(k == 0),
                    stop=(k == 1),
                )
            nc.vector.tensor_copy(qkT[f][:, W * tck:W * (tck + 1)], ps[:])
        for tt in range(WT * tck, WT * (tck + 1)):
            ps = ps_sm.tile([128, C], FP32, name="v_ps", tag="sm")
            for k in range(2):
                nc.tensor.matmul(
                    ps[:],
                    xTt[k][:, 128 * tt:128 * (tt + 1)],
                    wa_bf[k][:, 2 * C:3 * C],
                    start=(k == 0),
                    stop=(k == 1),
                )
            va = vaugp.tile([128, H * DA], BF16, name="va")
            nc.gpsimd.memset(va[:], 1.0)
            nc.vector.tensor_copy(
                va[:].rearrange("p (h d) -> p h d", h=H)[:, :, 0:D],
                ps[:].rearrange("p (h d) -> p h d", h=H),
            )
            vaug[tt] = va

    if taps is not None:
        for f in range(4):
            nc.gpsimd.dma_start(taps[f"qkT{f}"], qkT[f][:])
        nc.gpsimd.dma_start(taps["vaug0"], vaug[0][:])

    # --- attention: head-pairs hg (heads 2hg, 2hg+1), software-pipelined ---
    yT = [ytp.tile([128, T], BF16, name=f"yT{g}") for g in range(2)]
    for mc in range(MCN):
        for hg in range(4):
            nn_count = WT * (mc + 1)
            nn_last = nn_count - 1
            y_ps = ps_y.tile([128, W], FP32, name="y_ps", tag="y")
            s_list = [None] * nn_count
            pt_list = [None] * nn_count

            def emit_scores(nn):
                j = nn - WT * mc
                off = max(0, 128 * j)
                s_ps = ps_s.tile([128, 2, W], FP32, name="s_ps", tag="s")
                s_list[nn] = (s_ps, off)
                for i in range(2):
                    h = 2 * hg + i
                    fq, r = h // 4, 32 * (h % 4)
                    nc.tensor.matmul(
                        s_ps[:, i, ds(off, W - off)],
                        qkT[2 + fq][ds(r, 32), 128 * nn:128 * (nn + 1)],
                        qkT[fq][ds(r, 32), W * mc + off:W * (mc + 1)],
                        start=True,
                        stop=True,
                        tile_position=(r, 0),
                        skip_group_check=True,
                    )

            def emit_softmax_pv(nn):
                s_ps, off = s_list[nn]
                pt = ptp.tile([128, 2, W], BF16, name="pt")
                pt_list[nn] = pt
                nc.scalar.activation(
                    pt[:, :, ds(off, W - off)],
                    s_ps[:, :, ds(off, W - off)],
                    mybir.ActivationFunctionType.Exp,
                    scale=SCALE,
                )
                if nn - WT * mc >= 0:
                    # triangular mask on DVE: lower latency than gpsimd in
                    # the exp->PV hop, and the normalize chain drains during
                    # the mask-free non-diagonal window of the next head-pair
                    blk = pt[:, :, ds(off, 128)]
                    nc.vector.tensor_mul(blk, blk, tri2[:, :, 0:128])
                for i in range(2):
                    h = 2 * hg + i
                    nc.tensor.matmul(
                        y_ps[ds(64 * i, DA), ds(off, W - off)],
                        vaug[nn][:, DA * h:DA * (h + 1)],
                        pt[:, i, ds(off, W - off)],
                        start=(nn == 0),
                        stop=(nn == nn_last),
                        tile_position=(0, 64 * i),
                        skip_group_check=True,
                    )

            # software pipeline: scores(nn+1) ahead of exp/PV(nn) so the
            # Tensor FIFO never blocks behind the ACT dependency
            emit_scores(0)
            for nn in range(nn_count):
                if nn + 1 < nn_count:
                    emit_scores(nn + 1)
                emit_softmax_pv(nn)
                if taps is not None and mc == 0 and hg == 0 and nn == 0:
                    nc.gpsimd.dma_start(taps["pt00"], pt_list[0][:])

            # normalize straight out of PSUM: reciprocal of the denominator
            # row, K=1 ones-matmul partition-broadcast into PSUM, one
            # multiply into yT.  (walrus requires SBUF operands of a
            # tensor_tensor to share the start partition; both value inputs
            # here are PSUM, so only yT counts.)
            # (custom-DVE reciprocal must read SBUF, not PSUM - copy first)
            # denominator rows parked at partitions 0 and 32 (engine APs must
            # start at a multiple of 32); one approx reciprocal covers both
            g = hg // 2
            den_sb = nrmp.tile([64, W], FP32, name="den_sb", tag="den")
            for i in range(2):
                nc.vector.tensor_copy(den_sb[ds(32 * i, 1), :],
                                      y_ps[ds(64 * i + D, 1), :])
            # one batched reciprocal for both heads (rows 1-31/33-63 unused;
            # custom-DVE approx ops race under Tile, plain InstReciprocal
            # is dependable)
            rcp = nrmp.tile([64, W], FP32, name="rcp", tag="rcp")
            nc.vector.reciprocal(rcp[:], den_sb[:])
            rcp_bf = [nrmp.tile([1, W], BF16, name=f"rcp_bf{i}",
                                tag=f"rcpbf{i}") for i in range(2)]
            for i in range(2):
                nc.vector.tensor_copy(rcp_bf[i][:], rcp[ds(32 * i, 1), :])
            for i in range(2):
                row = 64 * (hg % 2) + 32 * i
                bc_ps = ps_sm.tile([D, W], FP32, name="bc_ps", tag="sm")
                nc.tensor.matmul(bc_ps[:], ones1[:], rcp_bf[i][:],
                                 start=True, stop=True)
                bcast = nrmp.tile([D, W], BF16, name="bcast", tag="bc")
                nc.vector.tensor_copy(bcast[:], bc_ps[:])
                nc.vector.tensor_mul(
                    yT[g][ds(row, D), W * mc:W * (mc + 1)],
                    y_ps[ds(64 * i, D), :],
                    bcast[:],
                )

        # projection for this m-chunk's t-tiles: its yT columns are final
        # once all four head-pairs are normalized; interleaving it here
        # shrinks the serial tail and feeds the PE at chunk boundaries
        for tt in range(WT * mc, WT * (mc + 1)):
            ps = ps_sm.tile([128, C], FP32, name="pj_ps", tag="sm")
            for g in range(2):
                nc.tensor.matmul(
                    ps[:],
                    yT[g][:, 128 * tt:128 * (tt + 1)],
                    wp_bf[g][:],
                    start=(g == 0),
                    stop=(g == 1),
                )
            ost = ostp.tile([128, C], FP32, name="ost")
            nc.vector.tensor_copy(ost[:], ps[:])
            # SWDGE: out-store waits are executed by Q7 ucode (no 1-wait cap)
            nc.gpsimd.dma_start(out[128 * tt:128 * (tt + 1), :], ost[:])

    if taps is not None:
        for g in range(2):
            nc.gpsimd.dma_start(taps[f"yT{g}"], yT[g][:])


def build_nc(T: int = 2048, debug: bool = False) -> bass.Bass:
    # Bacc (not raw Bass): its compile() pass legalizes multi-sem waits via
    # event semaphores - walrus only accepts one sem wait per instruction.
    nc = bacc.Bacc("TRN2", target_bir_lowering=False, debug=False,
                   num_devices=N_CORES)
    xT_d = nc.dram_tensor("xT", [C, T], BF16, kind="ExternalInput")
    wa_d = nc.dram_tensor("w_attn", [C, 3 * C], BF16, kind="ExternalInput")
    wp_d = nc.dram_tensor("w_proj", [C, C], BF16, kind="ExternalInput")
    out_d = nc.dram_tensor("out", [T, C], FP32, kind="ExternalOutput")
    taps = None
    if debug:
        taps = {}
        for f in range(4):
            taps[f"qkT{f}"] = nc.dram_tensor(
                f"tap_qkT{f}", [128, T], BF16, kind="ExternalOutput").ap()
        taps["vaug0"] = nc.dram_tensor(
            "tap_vaug0", [128, H * DA], BF16, kind="ExternalOutput").ap()
        taps["pt00"] = nc.dram_tensor(
            "tap_pt00", [128, 2 * min(512, T)], BF16,
            kind="ExternalOutput").ap()
        for g in range(2):
            taps[f"yT{g}"] = nc.dram_tensor(
                f"tap_yT{g}", [128, T], BF16, kind="ExternalOutput").ap()
    with tile.TileContext(nc) as tc:
        with ExitStack() as ctx:
            build_body(ctx, tc, xT_d.ap(), wa_d.ap(), wp_d.ap(), out_d.ap(), T,
                       taps=taps)
    nc.compile()
    return nc


_NC_CACHE: dict[int, bass.Bass] = {}


def _get_nc(T: int) -> bass.Bass:
    if T not in _NC_CACHE:
        _NC_CACHE[T] = build_nc(T)
    return _NC_CACHE[T]


def kernel(x: np.ndarray, w_attn: np.ndarray, w_proj: np.ndarray,
           _return_raw: bool = False, **run_kwargs) -> np.ndarray:
    import ml_dtypes
    B, T, C_ = x.shape
    assert B == N_CORES and C_ == C
    nc = _get_nc(T)
    wa = np.ascontiguousarray(np.asarray(w_attn, np.float32).astype(
        ml_dtypes.bfloat16))
    wp = np.ascontiguousarray(np.asarray(w_proj, np.float32).astype(
        ml_dtypes.bfloat16))
    xb = np.asarray(x, np.float32).astype(ml_dtypes.bfloat16)
    in_maps = [
        {"xT": np.ascontiguousarray(xb[b].T), "w_attn": wa, "w_proj": wp}
        for b in range(B)
    ]
    res = run_bass_kernel_spmd(nc, in_maps, list(range(N_CORES)), **run_kwargs)
    if _return_raw:
        return res
    out = np.stack([res.results[b]["out"] for b in range(B)])
    return out.astype(np.float32)
